# revision 1
# baseline (speedup 1.0000x reference)
"""Contrastive patch loss (InfoNCE over sampled voxel patches) on 8 TRN2 NeuronCores.

Math
----
Reference computes, per patch p and batch b, cs[k,l] = <t2n[:,i_pk], t1n[:,i_pl]>
over k=512 sampled voxels (i = idx[p]), e = exp(cs/bw), then the mean over
(p,b,j) of -log(0.5*e_jj*(1/colsum_j + 1/rowsum_j) + eps).

Since every sampled voxel index lives in [0, 512), cs is just a gather of the
512x512 Gram matrix G_b = t2n^T @ t1n:  cs[k,l] = G_b[i_k, i_l].  With
E_b = exp(G_b/bw) and c_p[s] = multiplicity of voxel s in patch p:

    rowsum_j = (E_b @ c_p)[i_j]        colsum_j = (E_b^T @ c_p)[i_j]
    pos_j    = diag(E_b)[i_j]

and the sum over j of any f(i_j) equals sum_s c_p[s] f(s).  So the whole
(P,B,K,K) tensor never needs to exist:

    loss = -1/(P*B*K) * sum_{b,p,s} c_p[s] *
           log(0.5*diagE_b[s]*(1/CS_b[s,p] + 1/RS_b[s,p]) + eps)

where RS_b = E_b @ C^T and CS_b = E_b^T @ C^T are (512, P) matmuls.

Sharding: 8 cores = 2 batches x 4 patch-quarters.  Each core builds E for its
batch (normalize, Gram matmul, exp), then computes RS/CS/loss terms for its 32
patches.  Per-core partial sums are returned as a (128,4) tile; the host adds
the 8 partials and applies -1/(P*B*K).

Precision: matmul operands are bf16 (verified: final rel err ~2e-5 vs f64);
accumulation, exp/log and the loss reduction stay fp32.
"""

import math

import ml_dtypes
import numpy as np

import concourse.bacc as bacc
import concourse.tile as tile
from concourse import hw_specs, mybir
from concourse.bass_utils import run_bass_kernel_spmd
from concourse.masks import make_identity

# Pin every ACTIVATE to the one table set that holds ln+exp+square+copy, so
# the kernel pays a single ACT_TABLE_LOAD instead of ping-ponging between the
# per-function default sets.  Indices (act_func_set_id) are preserved; only
# membership of the other sets is hidden from the placement pass.
_PIN_SET = "natural_log_exp_and_others"
_orig_get_tables = hw_specs.get_activation_tables


def _pinned_tables(arch):
    tabs = _orig_get_tables(arch)
    return {k: (v if k == _PIN_SET else set()) for k, v in tabs.items()}


bacc.get_activation_tables = _pinned_tables

B, C, S = 2, 256, 512
P, K = 128, 512
BW = 0.05
EPS = 1e-5
N_CORES = 8
PQ = P // 4  # patches per core (32)
F32 = mybir.dt.float32
BF16 = mybir.dt.bfloat16
USE_VREC = True  # vector.reciprocal; False -> exp(-ln(x)) on ScalarE


def _build_program():
    nc = bacc.Bacc("TRN2", target_bir_lowering=False, debug=False, num_devices=N_CORES)

    feat2 = nc.dram_tensor("feat2", [C, S], BF16, kind="ExternalInput")
    feat1 = nc.dram_tensor("feat1", [C, S], BF16, kind="ExternalInput")
    cntp = nc.dram_tensor("cntp", [128, 128], BF16, kind="ExternalInput")
    partial = nc.dram_tensor("partial", [128, 4], F32, kind="ExternalOutput")

    with tile.TileContext(nc) as tc:
        with (
            tc.tile_pool(name="const", bufs=1) as const,
            tc.tile_pool(name="feat", bufs=1) as featp,
            tc.tile_pool(name="big", bufs=1) as big,
            tc.tile_pool(name="tmp", bufs=2) as tmp,
            tc.tile_pool(name="small", bufs=2) as small,
            tc.tile_pool(name="ps", bufs=2, space="PSUM") as ps,
            tc.tile_pool(name="ps_small", bufs=2, space="PSUM") as ps_small,
        ):
            ident_bf = const.tile([128, 128], BF16, name="ident_bf", tag="ident_bf")
            make_identity(nc, ident_bf)
            one_1x1 = const.tile([1, 1], F32, name="one11", tag="one11")
            nc.vector.memset(one_1x1, 1.0)
            ones_col_bf = const.tile([128, 1], BF16, name="ones_col_bf", tag="ocb")
            nc.vector.memset(ones_col_bf, 1.0)
            ones_row = const.tile([1, 128], BF16, name="ones_row", tag="ones_row")
            nc.vector.memset(ones_row, 1.0)
            eps_col = const.tile([128, 1], F32, name="eps_col", tag="eps_col")
            nc.vector.memset(eps_col, EPS)
            ln_ibw_col = const.tile([128, 1], F32, name="ln_ibw_col", tag="lbc")
            nc.vector.memset(ln_ibw_col, math.log(1.0 / BW))

            # One DMA per tensor: features as (128, 2, S) [q, c-chunk, s],
            # counts pre-packed on host as (128, 128) [q, 32t+p].
            f2t = featp.tile([128, 2, S], BF16, name="f2t", tag="f2t")
            f1t = featp.tile([128, 2, S], BF16, name="f1t", tag="f1t")
            nc.sync.dma_start(
                out=f2t, in_=feat2[:, :].rearrange("(i q) s -> q i s", q=128)
            )
            nc.sync.dma_start(
                out=f1t, in_=feat1[:, :].rearrange("(i q) s -> q i s", q=128)
            )
            f2b = [f2t[:, i, :] for i in range(2)]
            f1b = [f1t[:, i, :] for i in range(2)]
            cnt_all = const.tile([128, 128], BF16, name="cnt_all", tag="cnt_all")
            nc.sync.dma_start(out=cnt_all, in_=cntp[:, :])
            cnt = [cnt_all[:, PQ * t : PQ * (t + 1)] for t in range(4)]
            wcnt = cnt_all

            # --- channel-wise sumsq via ones-matmul on bf16 squares ---
            ss_ps = []
            for nm, fch in (("2", f2b), ("1", f1b)):
                ssp = ps_small.tile([1, S], F32, name=f"ss_ps{nm}", tag="sm_ps_a")
                for i in range(2):
                    sq = tmp.tile([128, S], BF16, name="sq", tag="sq")
                    nc.scalar.activation(
                        out=sq, in_=fch[i],
                        func=mybir.ActivationFunctionType.Square,
                    )
                    nc.tensor.matmul(
                        out=ssp, lhsT=ones_col_bf, rhs=sq,
                        start=(i == 0), stop=(i == 1),
                    )
                ss_ps.append(ssp)
            ss2_ps, ss1_ps = ss_ps

            # ln(ss); keep both Ln calls adjacent (one ACT table set visit)
            lns2 = small.tile([1, S], F32, name="lns2", tag="lns2")
            nc.scalar.activation(
                out=lns2, in_=ss2_ps, func=mybir.ActivationFunctionType.Ln
            )
            lns1 = small.tile([1, S], F32, name="lns1", tag="lns1")
            nc.scalar.activation(
                out=lns1, in_=ss1_ps, func=mybir.ActivationFunctionType.Ln
            )

            # inv1 row: 1/sqrt(ss1) = exp(-0.5*ln(ss1))
            inv1_row = small.tile([1, S], BF16, name="inv1_row", tag="inv1_row")
            nc.scalar.activation(
                out=inv1_row, in_=lns1,
                func=mybir.ActivationFunctionType.Exp, scale=-0.5,
            )

            # inv2/BW in column form: transpose ln(ss2) then
            # exp(-0.5*ln(ss) + ln(1/BW)) in one activation
            c2_ps = ps_small.tile([128, 4], F32, name="c2_ps", tag="sm_ps_b")
            for m in range(4):
                nc.tensor.transpose(
                    out=c2_ps[:, m : m + 1],
                    in_=lns2[0:1, 128 * m : 128 * (m + 1)],
                    identity=one_1x1,
                )
            inv2bw = small.tile([128, 4], F32, name="inv2bw", tag="inv2bw")
            nc.scalar.activation(
                out=inv2bw, in_=c2_ps,
                func=mybir.ActivationFunctionType.Exp,
                scale=-0.5, bias=ln_ibw_col,
            )

            # --- normalized t1 in bf16: f1nb = f1b * bcast(inv1) ---
            bc_ps = ps.tile([128, S], F32, name="bc_ps", tag="big_ps")
            nc.tensor.matmul(out=bc_ps, lhsT=ones_row, rhs=inv1_row)
            bc = tmp.tile([128, S], F32, name="bc", tag="bc")
            nc.any.tensor_copy(out=bc, in_=bc_ps)

            # --- G = t2_raw^T @ t1_raw (bf16), col-scale by inv1, exp with
            # inv2/BW row scale ---
            e = [big.tile([128, S], BF16, name=f"e_{m}", tag=f"e_{m}") for m in range(4)]
            for m in range(4):
                g_ps = ps.tile([128, S], F32, name="g_ps", tag="big_ps")
                msl = slice(128 * m, 128 * (m + 1))
                for i in range(2):
                    nc.tensor.matmul(
                        out=g_ps, lhsT=f2b[i][:, msl], rhs=f1b[i],
                        start=(i == 0), stop=(i == 1),
                    )
                gsc = tmp.tile([128, S], F32, name="gsc", tag="gsc")
                nc.vector.tensor_tensor(
                    out=gsc, in0=g_ps, in1=bc, op=mybir.AluOpType.mult
                )
                nc.scalar.activation(
                    out=e[m], in_=gsc,
                    func=mybir.ActivationFunctionType.Exp,
                    scale=inv2bw[:, m : m + 1],
                )

            # --- E^T via 16 PE transposes (bf16) ---
            et = [big.tile([128, S], BF16, name=f"et_{a}", tag=f"et_{a}") for a in range(4)]
            for a in range(4):
                et_ps = ps.tile([128, S], BF16, name="et_ps", tag="big_ps")
                asl = slice(128 * a, 128 * (a + 1))
                for m in range(4):
                    nc.tensor.transpose(
                        out=et_ps[:, 128 * m : 128 * (m + 1)],
                        in_=e[m][:, asl],
                        identity=ident_bf,
                    )
                nc.any.tensor_copy(out=et[a], in_=et_ps)

            # --- diag(E) in column form ---
            dcol = small.tile([128, 4], F32, name="dcol", tag="dcol")
            for m in range(4):
                scr = tmp.tile([128, 128], F32, name="scr_diag", tag="scr_diag")
                nc.vector.tensor_tensor(
                    out=scr,
                    in0=e[m][:, 128 * m : 128 * (m + 1)],
                    in1=ident_bf,
                    op=mybir.AluOpType.mult,
                )
                nc.vector.tensor_reduce(
                    out=dcol[:, m : m + 1], in_=scr,
                    axis=mybir.AxisListType.X, op=mybir.AluOpType.add,
                )

            # --- RS/CS matmuls + loss terms per 128-row block of s ---
            acc = small.tile([128, 4], F32, name="acc", tag="acc")
            for m in range(4):
                msl = slice(128 * m, 128 * (m + 1))
                rs_ps = ps_small.tile([128, PQ], F32, name="rs_ps", tag="sm_ps_a")
                cs_ps = ps_small.tile([128, PQ], F32, name="cs_ps", tag="sm_ps_b")
                for t in range(4):
                    nc.tensor.matmul(
                        out=rs_ps, lhsT=et[t][:, msl], rhs=cnt[t],
                        start=(t == 0), stop=(t == 3),
                    )
                for t in range(4):
                    nc.tensor.matmul(
                        out=cs_ps, lhsT=e[t][:, msl], rhs=cnt[t],
                        start=(t == 0), stop=(t == 3),
                    )
                rinv = small.tile([128, PQ], F32, name="rinv", tag="rinv")
                cinv = small.tile([128, PQ], F32, name="cinv", tag="cinv")
                nc.vector.reciprocal(out=rinv, in_=rs_ps)
                nc.vector.reciprocal(out=cinv, in_=cs_ps)
                ssum = small.tile([128, PQ], F32, name="ssum", tag="ssum")
                nc.vector.tensor_tensor(
                    out=ssum, in0=rinv, in1=cinv, op=mybir.AluOpType.add
                )
                x = small.tile([128, PQ], F32, name="x", tag="x")
                nc.vector.tensor_scalar_mul(out=x, in0=ssum, scalar1=dcol[:, m : m + 1])
                g = small.tile([128, PQ], F32, name="g", tag="g")
                nc.scalar.activation(
                    out=g, in_=x,
                    func=mybir.ActivationFunctionType.Ln, scale=0.5, bias=eps_col,
                )
                scr2 = small.tile([128, PQ], F32, name="scr2", tag="scr2")
                nc.vector.tensor_tensor(
                    out=scr2, in0=g, in1=cnt[m], op=mybir.AluOpType.mult
                )
                nc.vector.tensor_reduce(
                    out=acc[:, m : m + 1], in_=scr2,
                    axis=mybir.AxisListType.X, op=mybir.AluOpType.add,
                )
            nc.sync.dma_start(out=partial[:, :], in_=acc)

    nc.compile()
    return nc


_NC = None


def _run(t2_feat, t1_feat, idx, trace=False, trace_kwargs=None):
    global _NC
    if _NC is None:
        _NC = _build_program()

    t2 = np.ascontiguousarray(np.asarray(t2_feat, np.float32).reshape(B, C, S))
    t1 = np.ascontiguousarray(np.asarray(t1_feat, np.float32).reshape(B, C, S))
    idx = np.asarray(idx)

    counts = np.zeros((P, S), np.float32)
    np.add.at(counts, (np.arange(P)[:, None], idx), 1.0)

    in_maps = []
    for core in range(N_CORES):
        b, q = divmod(core, 4)
        in_maps.append(
            {
                "feat2": t2[b].astype(ml_dtypes.bfloat16),
                "feat1": t1[b].astype(ml_dtypes.bfloat16),
                "cntp": np.ascontiguousarray(
                    counts[PQ * q : PQ * (q + 1)]
                    .T.reshape(4, 128, PQ)
                    .transpose(1, 0, 2)
                    .reshape(128, 128)
                ).astype(ml_dtypes.bfloat16),
            }
        )

    kwargs = {}
    if trace:
        kwargs = dict(trace=True, trace_kwargs=trace_kwargs or {})
    res = run_bass_kernel_spmd(_NC, in_maps, core_ids=list(range(N_CORES)), **kwargs)
    total = sum(r["partial"].sum(dtype=np.float64) for r in res.results)
    loss = -total / (P * B * K)
    return np.array(loss, dtype=np.float32), res


def kernel(t2_feat, t1_feat, idx):
    out, _ = _run(t2_feat, t1_feat, idx)
    return out

